# revision 32
# baseline (speedup 1.0000x reference)
"""Table-batched INT8 embedding-bag kernel for Trainium2 (8 NeuronCores).

Table-wise model parallel: core t holds table t as f16 rows (256B), declared
f32[E_pad, 64] for the DMA layer. Samples are binned by 32768-row table
segment (int16 dma_gather index space) and packed into 128-sample chunks
whose bags fall in a narrow window [wc, wc+W) shared across all 8 tables,
so the SPMD program is identical on every core.

The gather is Q7-descriptor-generation bound (~9 ns/row per Q7 core pair),
so gathers round-robin over 4 SWDGE queues: queue q's descriptors are
generated by Q7 cores (2q, 2q+1), quadrupling descriptor throughput
(measured 2253 us -> ~480 us end to end). Each queue's idx stream is
packed into its pair's 32 partitions only (2x replication instead of 8x).
dma_gather is capped at 1024 idxs/instruction by the SWDGE descriptor-ring
capacity (>= 81 descs/engine hangs the ring). Each granule's emptiest
chunk is placed last with trailing -1 idxs (skipped by the ucode, >= 1
real idx kept so every core emits the ring-booked descriptor count).

Each chunk is pooled on the PE as psum[:, wc:wc+W] += chunk_rows^T @ mask;
the [128, W]-per-chunk mask (one w*s weight per sample at its bag offset)
is constructed on-chip by the DVE from per-slot (relbag, w*s) pairs via
is_equal against an iota row + multiply, replacing the 13 MB host mask
stream with 0.9 MB. A [128, 4096] f32 PSUM accumulator collects
everything; the tail converts to f16, DMA-transposes 128x128 blocks to
[bag, d], adds the host-computed per-bag bias sum (sum_j w*b), and stores
the [B, D] shard. The host reassembles [B, T*D].
"""

import os
import sys

sys.path.insert(0, "/opt/trn_rl_repo")

import numpy as np

import concourse.bacc as bacc
import concourse.bass as bass
import concourse.mybir as mybir
import concourse.tile as tile
from concourse import bass_utils, library_config
from concourse._compat import cdiv

T, E, D = 8, 250000, 128
B, L = 4096, 50
P = 128
SEG_ROWS = 32768
NSEG = cdiv(E, SEG_ROWS)            # 8 segments of int16-addressable rows
E_PAD = NSEG * SEG_ROWS             # 262144
EW = 64                              # f32 words per 256B row (128 x f16)
W = int(os.environ.get("BASS_W", "32"))  # bag-window width per chunk
BANK_BAGS = 512                      # 2KB psum bank / 4B f32
# chunks per dma_gather; 16 => 2048 idxs = 129 descs/engine (ring cap 256)
GMAX = int(os.environ.get("BASS_GMAX", "8"))
TAIL_GROUPS = [(0, 4), (4, 7), (7, 8)]   # psum-bank ranges per tail group

f16 = mybir.dt.float16
f32 = mybir.dt.float32
i16 = mybir.dt.int16

add = mybir.AluOpType.add


# ---------------------------------------------------------------- planner ---

def plan_chunks(bag_lists):
    """bag_lists[t][k]: sorted bag ids (one per sample) of table t, segment k.
    Greedy shared windows: wc = min over tables of next unassigned bag
    (shifted left off psum-bank crossings); each table takes up to 128
    samples with bag < wc+W.  Returns per-segment chunk counts, window
    starts, and per-(t,k,chunk) sample counts.
    """
    seg_chunks = []
    wcs = []
    takes = []            # list over segments: [nchunk, T] sample counts
    for k in range(NSEG):
        bl = [bag_lists[t][k] for t in range(T)]
        n = [len(x) for x in bl]
        ptr = [0] * T
        seg_takes = []
        while True:
            nxt = [bl[t][ptr[t]] for t in range(T) if ptr[t] < n[t]]
            if not nxt:
                break
            wc = int(min(nxt))
            lim = BANK_BAGS - W
            if wc % BANK_BAGS > lim:
                wc -= wc % BANK_BAGS - lim
            wc = min(wc, B - W)
            row = np.zeros(T, np.int32)
            for t in range(T):
                hi = np.searchsorted(bl[t], wc + W, side="left")
                take = min(P, hi - ptr[t])
                row[t] = take
                ptr[t] += take
            wcs.append(wc)
            seg_takes.append(row)
        seg_chunks.append(len(seg_takes))
        takes.append(np.array(seg_takes).reshape(-1, T))
    return seg_chunks, np.asarray(wcs, np.int32), takes


def host_prep(indices, per_sample_weights, weights_q, scales, biases):
    indices = np.asarray(indices).astype(np.int64).reshape(T, B * L)
    psw = np.asarray(per_sample_weights, dtype=np.float32).reshape(T, B * L)
    weights_q = np.asarray(weights_q, dtype=np.uint8)
    scales = np.asarray(scales, dtype=np.float32)
    biases = np.asarray(biases, dtype=np.float32)

    # f16 table rows, padded to E_PAD, declared f32[:, 64] for the DMA layer
    pack = np.zeros((T, E_PAD, D), np.float16)
    pack[:, :E, :] = weights_q.astype(np.float16)
    pack_f32 = pack.reshape(T, E_PAD * D).view(np.float32).reshape(T, E_PAD, EW)

    ws = (psw * np.take_along_axis(scales, indices, axis=1)).astype(np.float16)
    wb = psw * np.take_along_axis(biases, indices, axis=1)
    bias_vec = wb.reshape(T, B, L).sum(axis=2).astype(np.float32)   # [T, B]

    seg = (indices >> 15).astype(np.int32)          # [T, B*L]
    bag = np.broadcast_to((np.arange(B * L) // L)[None, :], (T, B * L))

    # per (t, k): sample ids ordered by (bag, j) = original order
    sample_ids = [[np.flatnonzero(seg[t] == k) for k in range(NSEG)]
                  for t in range(T)]
    bag_lists = [[bag[t][s] for s in sample_ids[t]] for t in range(T)]

    seg_chunks, wcs, takes = plan_chunks(bag_lists)
    total_chunks = int(sum(seg_chunks))
    chunk_base = np.cumsum([0] + seg_chunks)

    # granules (<=GMAX consecutive planner chunks of one segment), merged
    # across segments by ascending window start; chunks renumbered into
    # processing order so the device consumes idx/mask columns linearly
    raw = []
    for k in range(NSEG):
        ck0, ck1 = int(chunk_base[k]), int(chunk_base[k + 1])
        for c0 in range(ck0, ck1, GMAX):
            gn = min(GMAX, ck1 - c0)
            raw.append((int(wcs[c0]), k, c0, gn))
    raw.sort()
    # within each granule, order chunks by total fill (descending) so the
    # emptiest chunk is last; its unfilled slots become trailing -1 idxs,
    # which the gather ucode skips (desc-gen + transfer savings)
    fill_sum = [takes[k].sum(axis=1) for k in range(NSEG)]
    proc_of_planner = np.empty(total_chunks, np.int64)
    last_planner = []          # per granule: planner chunk placed last
    pos = 0
    for _, k, c0, gn in raw:
        loc = np.arange(c0, c0 + gn) - chunk_base[k]
        order = np.argsort(-fill_sum[k][loc], kind="stable")
        for j, o in enumerate(order):
            proc_of_planner[c0 + o] = pos + j
        last_planner.append(int(c0 + order[-1]))
        pos += gn
    wcs_proc = np.empty(total_chunks, np.int32)
    wcs_proc[proc_of_planner] = wcs

    # queue-packed idx: granule gi runs on queue q=gi%NUM_Q whose Q7 pair
    # reads partitions [32q, 32q+32) only, so 4 granules share columns.
    # Each pair needs 2 copies of the [16, gn*8] idx block (even/odd core).
    colq = []
    qcur = [0] * NUM_Q
    for gi, (_, k, c0, gn) in enumerate(raw):
        q = gi % NUM_Q
        colq.append(qcur[q])
        qcur[q] += gn
    ncols = max(qcur)
    granules = [(k, gn, gi % NUM_Q, colq[gi])
                for gi, (_, k, c0, gn) in enumerate(raw)]
    # proc chunk -> (queue, queue-local column)
    q_of_proc = np.empty(total_chunks, np.int64)
    qcol_of_proc = np.empty(total_chunks, np.int64)
    pos = 0
    for gi, (_, k, c0, gn) in enumerate(raw):
        q_of_proc[pos:pos + gn] = gi % NUM_Q
        qcol_of_proc[pos:pos + gn] = colq[gi] + np.arange(gn)
        pos += gn

    idxq = np.zeros((T, P, ncols * 8), np.int16)
    # per-slot window offset and weight; device builds the dense mask
    relb = np.zeros((T, P, total_chunks), np.float16)
    wval = np.zeros((T, P, total_chunks), np.float16)

    for t in range(T):
        for k in range(NSEG):
            s_all = sample_ids[t][k]
            tk = takes[k][:, t]                      # samples per chunk
            cum = np.cumsum(tk)
            assert cum[-1] == len(s_all)
            nck = seg_chunks[k]
            # chunk id and slot per sample (samples are consumed in order)
            c_local = np.repeat(np.arange(nck), tk)
            slot = np.arange(len(s_all)) - np.repeat(cum - tk, tk)
            cc = chunk_base[k] + c_local
            ccp = proc_of_planner[cc]
            rows_local = (indices[t, s_all] & (SEG_ROWS - 1)).astype(np.int16)
            qp = 32 * q_of_proc[ccp]
            qc = qcol_of_proc[ccp]
            idxq[t, qp + slot % 16, qc * 8 + slot // 16] = rows_local
            idxq[t, qp + 16 + slot % 16, qc * 8 + slot // 16] = rows_local
            rel = bag[t][s_all] - wcs[cc]
            assert rel.min() >= 0 and rel.max() < W
            relb[t, slot, ccp] = rel.astype(np.float16)
            wval[t, slot, ccp] = ws[t, s_all]

    # trailing -1 idxs on each granule's last chunk (per-table fill)
    slot_all = np.arange(P)
    if os.environ.get("BASS_TRIM", "1") == "0":
        slot_all = slot_all[:0]
    for gi, (_, k, c0, gn) in enumerate(raw):
        pl = last_planner[gi]
        qp = 32 * (gi % NUM_Q)
        qc = colq[gi] + gn - 1
        for t in range(T):
            # keep slot 0 un-trimmed: every core must emit the same
            # descriptor-group count as the decode-side ring booking
            f = max(int(takes[k][pl - chunk_base[k], t]), 1)
            if f < P:
                sl = slot_all[f:]
                idxq[t, qp + sl % 16, qc * 8 + sl // 16] = -1
                idxq[t, qp + 16 + sl % 16, qc * 8 + sl // 16] = -1

    jconst = np.broadcast_to(np.arange(W, dtype=np.float16)[None, :],
                             (P, W)).copy()
    return (pack_f32, idxq, relb, wval, jconst, bias_vec,
            granules, wcs_proc.tolist(), total_chunks, ncols)


# ----------------------------------------------------------------- device ---

MASK_GRAN = int(os.environ.get("BASS_MASK_GRAN", str(max(1, 96 // GMAX))))
EARLY_TAIL = True     # interleave tail with gather stream
NUM_Q = int(os.environ.get("BASS_NUM_Q", "4"))  # SWDGE queues (Q7 core pairs)


SCRATCH = int(os.environ.get("BASS_SCRATCH", "16384"))


def build_nc(granules, wcs, total_chunks, ncols, reps=1):
    nc = bacc.Bacc("TRN2", target_bir_lowering=False, debug=False,
                   enable_asserts=False, num_devices=8,
                   num_swdge_queues=NUM_Q,
                   dynamic_dma_scratch_size=SCRATCH)
    packd = nc.dram_tensor("pack", [E_PAD, EW], f32, kind="ExternalInput")
    idxd = nc.dram_tensor("idx", [P, ncols * 8], i16, kind="ExternalInput")
    relbd = nc.dram_tensor("relb", [P, total_chunks], f16,
                           kind="ExternalInput")
    wvald = nc.dram_tensor("wval", [P, total_chunks], f16,
                           kind="ExternalInput")
    jconstd = nc.dram_tensor("jconst", [P, W], f16, kind="ExternalInput")
    outd = nc.dram_tensor("out", [B, D], f16, kind="ExternalOutput")

    ng = len(granules)
    gstart = np.cumsum([0] + [gn for _, gn, _, _ in granules])
    # bag high-water: min window start among granules >= i (chunks within a
    # granule are fill-ordered, so take the min over the whole granule)
    rest_min = np.full(ng + 1, B, np.int64)
    for i in range(ng - 1, -1, -1):
        gmin = min(int(wcs[c]) for c in range(int(gstart[i]),
                                              int(gstart[i + 1])))
        rest_min[i] = min(rest_min[i + 1], gmin)

    gat_bufs = max(4, 96 // GMAX)
    with tile.TileContext(nc) as tc:
        with (
            tc.tile_pool(name="gat", bufs=gat_bufs) as gp,
            tc.tile_pool(name="fin", bufs=3) as fp,
            tc.tile_pool(name="persist", bufs=1) as pp,
            tc.tile_pool(name="idxp", bufs=1) as ip,
            tc.tile_pool(name="mskp", bufs=3) as mp,
            tc.tile_pool(name="ps", bufs=1, space="PSUM") as psp,
        ):
            nc.gpsimd.load_library(library_config.mlp)

            accs = [psp.tile([P, BANK_BAGS], f32, tag=f"acc{j}",
                             name=f"acc{j}")
                    for j in range(B // BANK_BAGS)]

            def emit_body(rep):
                for j, a in enumerate(accs):
                    nc.vector.memset(a[:], 0.0)

                next_tail = 0
                state = {"rr": 0, "nt": 0}

                def emit_tail(j):                # j: tail group id
                    rr = state["rr"]
                    state["rr"] += 1
                    b0, b1 = TAIL_GROUPS[j]
                    grp = (b1 - b0) * BANK_BAGS
                    tpg = grp // P
                    st = fp.tile([P, grp], f16, tag="st", name=f"st{j}_{rep}")
                    for i, jb in enumerate(range(b0, b1)):
                        sl = slice(i * BANK_BAGS, (i + 1) * BANK_BAGS)
                        if (rr + i) % 2 == 0:
                            nc.vector.tensor_copy(out=st[:, sl], in_=accs[jb][:])
                        else:
                            nc.scalar.copy(st[:, sl], accs[jb][:])
                    trT = fp.tile([P, grp], f16, tag="trT", name=f"trT{j}_{rep}")
                    eng = (nc.sync, nc.scalar)[rr % 2]
                    eng.dma_start_transpose(
                        trT[:].rearrange("b (t d) -> b t d", d=P), st[:])
                    eng2 = (nc.scalar, nc.sync)[rr % 2]
                    eng2.dma_start(
                        outd.ap()[b0 * BANK_BAGS:b1 * BANK_BAGS, :]
                        .rearrange("(t b) d -> b t d", t=tpg),
                        trT[:])

                # whole idx tensor resident (queue-packed); relb/wval/jconst
                # load upfront; dense mask is constructed on-chip (DVE).
                n_mp = cdiv(ng, MASK_GRAN)

                rb = pp.tile([P, total_chunks], f16, tag="rb",
                             name=f"rb_{rep}")
                wv = pp.tile([P, total_chunks], f16, tag="wv",
                             name=f"wv_{rep}")
                jt = pp.tile([P, W], f16, tag="jt", name=f"jt_{rep}")
                nc.scalar.dma_start(rb[:], relbd.ap()[:, :])
                nc.scalar.dma_start(wv[:], wvald.ap()[:, :])
                nc.scalar.dma_start(jt[:], jconstd.ap()[:, :])

                idxt = ip.tile([P, ncols * 8], i16, tag="idx",
                               name=f"idx_{rep}")
                nc.sync.dma_start(idxt[:], idxd.ap()[:, :])

                def mask_build(pi):
                    gi0 = pi * MASK_GRAN
                    c0 = int(gstart[gi0])
                    c1 = int(gstart[min(gi0 + MASK_GRAN, ng)])
                    nch = c1 - c0
                    mt = mp.tile([P, nch * W], f16, tag="m",
                                 name=f"m{gi0}_{rep}")
                    mv = mt[:].rearrange("p (c w) -> p c w", w=W)
                    rb_sl = rb[:, c0:c1]
                    rbv = bass.AP(rb_sl.tensor, rb_sl.offset,
                                  list(rb_sl.ap) + [(0, W)])
                    j_sl = jt[:]
                    jv = bass.AP(j_sl.tensor, j_sl.offset,
                                 [j_sl.ap[0], (0, nch), j_sl.ap[1]])
                    wv_sl = wv[:, c0:c1]
                    wvv = bass.AP(wv_sl.tensor, wv_sl.offset,
                                  list(wv_sl.ap) + [(0, W)])
                    nc.vector.tensor_tensor(out=mv, in0=rbv, in1=jv,
                                            op=mybir.AluOpType.is_equal)
                    nc.vector.tensor_tensor(out=mv, in0=mv, in1=wvv,
                                            op=mybir.AluOpType.mult)
                    return mt, c0

                if rep == 0:
                    # gather buffers may be left partially unwritten by the
                    # trailing-idx skip; zero them once so stale SBUF bits
                    # can't inject NaN into the (mask=0) matmul lanes
                    for _ in range(gat_bufs):
                        z = gp.tile([P, GMAX * EW], f32, tag="g")
                        nc.vector.memset(z[:], 0.0)

                m, m_base = mask_build(0)
                nxt = mask_build(1) if n_mp > 1 else None
                for gi, (k, gn, q, col) in enumerate(granules):
                    if gi % MASK_GRAN == 0 and gi > 0:
                        m, m_base = nxt
                        pi = gi // MASK_GRAN
                        nxt = mask_build(pi + 1) if pi + 1 < n_mp else None

                    cg = int(gstart[gi])
                    g = gp.tile([P, gn * EW], f32, tag="g")
                    nc.gpsimd.dma_gather(
                        out_ap=g[:].rearrange("p (c r) -> p c r", r=EW),
                        in_ap=packd.ap()[k * SEG_ROWS:(k + 1) * SEG_ROWS, :],
                        idxs_ap=idxt[:, col * 8:(col + gn) * 8],
                        num_idxs=gn * P,
                        num_idxs_reg=gn * P,
                        elem_size=EW,
                        queue_num=q,
                    )
                    g16 = g[:].bitcast(f16)
                    for c in range(gn):
                        wc = int(wcs[cg + c])
                        j = wc // BANK_BAGS
                        lo = wc - j * BANK_BAGS
                        ml = cg + c - m_base
                        nc.tensor.matmul(
                            out=accs[j][:, lo:lo + W],
                            lhsT=g16[:, c * D:(c + 1) * D],
                            rhs=m[:, ml * W:(ml + 1) * W],
                            start=False, stop=True,
                            skip_group_check=True,
                        )
                    # psum banks fully below every remaining window can go now
                    while (EARLY_TAIL and next_tail < len(TAIL_GROUPS)
                           and TAIL_GROUPS[next_tail][1] * BANK_BAGS
                               <= rest_min[gi + 1]):
                        emit_tail(next_tail)
                        next_tail += 1
                while next_tail < len(TAIL_GROUPS):
                    emit_tail(next_tail)
                    next_tail += 1

            for rep in range(reps):
                emit_body(rep)
    nc.compile()
    return nc


_NC_CACHE = {}


def get_nc(granules, wcs, total_chunks, ncols, reps=None):
    if reps is None:
        reps = int(os.environ.get("BASS_KERNEL_REPS", "1"))
    key = (tuple(granules), tuple(wcs), ncols, reps)
    if key not in _NC_CACHE:
        _NC_CACHE[key] = build_nc(granules, wcs, total_chunks, ncols,
                                  reps=reps)
    return _NC_CACHE[key]


def kernel(indices, offsets, per_sample_weights, weights_q, scales, biases,
           **run_kwargs):
    (pack_f32, idx_rep, relb, wval, jconst, bias_vec,
     granules, wcs, total_chunks, ncols) = host_prep(
        indices, per_sample_weights, weights_q, scales, biases)
    nc = get_nc(granules, wcs, total_chunks, ncols)
    in_maps = [
        {
            "pack": pack_f32[t],
            "idx": idx_rep[t],
            "relb": relb[t],
            "wval": wval[t],
            "jconst": jconst,
        }
        for t in range(T)
    ]
    res = bass_utils.run_bass_kernel_spmd(
        nc, in_maps, core_ids=list(range(T)), **run_kwargs
    )
    out = np.empty((B, T * D), np.float16)
    for t in range(T):
        out[:, t * D:(t + 1) * D] = (
            res.results[t]["out"].astype(np.float32) + bias_vec[t][:, None]
        ).astype(np.float16)
    kernel.last_result = res
    return out

